# revision 8
# baseline (speedup 1.0000x reference)
"""Batched TGCN (GCN-GRU) Trainium2 kernel — packed-layout v3.

Strategy:
  - GCNConv linearity: agg = (A_norm^T X) over all B*Fin*T = 96 feature columns,
    computed ONCE (stage 1), shared by all 3 gates and 12 timesteps.
  - A streamed as fp8e3 (x64 scale, compensated by x/64 in bf16) -> half DMA.
  - Stage 2 GRU in a PACKED layout [128, 2500]: partition 64h+f holds feature f
    of batch-pair h (b = 2h + b2), column c = 1250*b2 + n_local. Every DVE/ACT
    op runs 128 partitions wide (engine cost is free-dim-driven). Gate matmuls
    use block-diagonal weights [W ⊕ W]; agg contribution + bias enter via a
    K=5 matmul (4 packed agg rows + a ones row) so sigmoid/tanh read fully
    biased PSUM and R|Z share ACT instructions.
  - x_feat is ordered fin-major (f = b*24 + fin*12 + t) so the packed agg
    tile [5, T, 2500] is filled by 8 large DMAs (no small-DMA stage gap that
    would let the PE clock re-throttle).
  - Stage 2 runs as 2 independent column streams of 1250 (software pipeline
    over the recurrence chain); lin output of step t is emitted after the zr
    matmuls of t+1 to fill PE dependency stalls. lin bias added on host.
  - 8 cores, node-sharded (1250 dst nodes/core), zero cross-core comms.
"""

import numpy as np
import ml_dtypes

import concourse.mybir as mybir
import concourse.tile as tile
from concourse import bacc
from concourse.bass import ds

F32 = mybir.dt.float32
BF16 = mybir.dt.bfloat16
FP8E3 = mybir.dt.float8e3
AF = mybir.ActivationFunctionType
ALU = mybir.AluOpType

# Problem constants (hardcoded per contract)
N_NODES = 10000
FIN = 2
HID = 64
OUT = 16
B = 4
T = 12
N_CORES = 8

P = 128
NPC = N_NODES // N_CORES          # 1250 dst nodes per core
NFEAT = B * FIN * T               # 96 aggregation features
N_SRC_PAD = ((N_NODES + P - 1) // P) * P   # 10112
KCH = N_SRC_PAD // P              # 79 k-chunks
KB = 8                            # k-chunks per A-stream DMA
A_SCALE = 64.0                    # fp8 pre-scale for A (power of 2, exact)
W = NPC * B // 2                  # 2500 packed columns
SW = W // 2                       # 1250 per stream
LG = W // 4                       # 625: lin column group width
CHUNKS = [(0, 512), (512, 512), (1024, 226)]   # per-stream psum chunks


def build_program():
    nc = bacc.Bacc("TRN2", target_bir_lowering=False, debug=False)

    a_t = nc.dram_tensor("a_t", [P, KCH, NPC], FP8E3, kind="ExternalInput")
    x_feat = nc.dram_tensor("x_feat", [P, KCH, NFEAT], BF16, kind="ExternalInput")
    lr2bd = nc.dram_tensor("lr2bd", [P, P], BF16, kind="ExternalInput")
    lz2bd = nc.dram_tensor("lz2bd", [P, P], BF16, kind="ExternalInput")
    lh2bd = nc.dram_tensor("lh2bd", [P, P], BF16, kind="ExternalInput")
    agr_d = nc.dram_tensor("agr", [5, P], BF16, kind="ExternalInput")
    agz_d = nc.dram_tensor("agz", [5, P], BF16, kind="ExternalInput")
    agh_d = nc.dram_tensor("agh", [5, P], BF16, kind="ExternalInput")
    wlin_d = nc.dram_tensor("wlin", [P, 32], BF16, kind="ExternalInput")
    ones_d = nc.dram_tensor("ones_d", [1, T * W], BF16, kind="ExternalInput")
    out_d = nc.dram_tensor("out", [T, P, LG], F32, kind="ExternalOutput")

    with tile.TileContext(nc) as tc:
        with tc.tile_pool(name="persist", bufs=1) as pp:
            x_sb = pp.tile([P, KCH, NFEAT], BF16, tag="x_sb")
            nc.sync.dma_start(x_sb[:], x_feat[:])
            wR = pp.tile([P, P], BF16, tag="wR")
            nc.sync.dma_start(wR[:], lr2bd[:])
            wZ = pp.tile([P, P], BF16, tag="wZ")
            nc.sync.dma_start(wZ[:], lz2bd[:])
            wH = pp.tile([P, P], BF16, tag="wH")
            nc.sync.dma_start(wH[:], lh2bd[:])
            aR = pp.tile([5, P], BF16, tag="aR")
            nc.sync.dma_start(aR[:], agr_d[:])
            aZ = pp.tile([5, P], BF16, tag="aZ")
            nc.sync.dma_start(aZ[:], agz_d[:])
            aH = pp.tile([5, P], BF16, tag="aH")
            nc.sync.dma_start(aH[:], agh_d[:])
            wL = pp.tile([P, 32], BF16, tag="wL")
            nc.sync.dma_start(wL[:], wlin_d[:])

            # packed agg [5, T, W]: row 2h+fin, row 4 = ones (bias path)
            aggp = pp.tile([5, T, W], BF16, tag="aggp")
            nc.sync.dma_start(aggp[4:5, :, :], ones_d[:])

            # ACT table warm (sigmoid set also holds tanh) + H0 = 0
            warm = pp.tile([1, 1], BF16, tag="warm")
            nc.scalar.activation(warm[:], wL[0:1, 0:1], AF.Sigmoid)
            h_bufs = [pp.tile([P, W], BF16, tag=f"h{i}", name=f"h{i}") for i in range(2)]
            nc.gpsimd.memset(h_bufs[0][:], 0.0)

            agg_nodes = pp.tile([NFEAT, NPC], BF16, tag="agg_nodes")

            # ---------------- Stage 1: aggregation ----------------
            dst_tiles = [(0, 512), (512, 512), (1024, 226)]
            with (
                tc.tile_pool(name="astream", bufs=3) as ap_,
                tc.tile_pool(name="apsum", bufs=1, space="PSUM") as aps,
            ):
                psums = [aps.tile([NFEAT, w], F32, tag=f"agp{i}", name=f"agps{i}")
                         for i, (_, w) in enumerate(dst_tiles)]
                n_ktiles = (KCH + KB - 1) // KB
                for kt in range(n_ktiles):
                    k0 = kt * KB
                    kb = min(KB, KCH - k0)
                    a_sb = ap_.tile([P, KB, NPC], FP8E3, tag="a_sb")
                    nc.sync.dma_start(a_sb[:, :kb], a_t[:, k0 : k0 + kb, :])
                    for kl in range(kb):
                        k = k0 + kl
                        for i, (doff, w) in enumerate(dst_tiles):
                            nc.tensor.matmul(
                                psums[i][:],
                                lhsT=x_sb[:, k, :],
                                rhs=a_sb[:, kl, ds(doff, w)],
                                start=(k == 0),
                                stop=(k == KCH - 1),
                            )
                for i, (doff, w) in enumerate(dst_tiles):
                    nc.vector.tensor_copy(agg_nodes[:, ds(doff, w)], psums[i][:])

            # aggp[2h+fin, t, b2*1250 + n] = agg_nodes[b*24 + fin*12 + t, n]
            # (x_feat is fin-major so one DMA per (b, fin) covers all t)
            for b in range(B):
                h, b2 = b // 2, b % 2
                for fin in range(FIN):
                    r = b * (FIN * T) + fin * T
                    nc.sync.dma_start(
                        aggp[2 * h + fin : 2 * h + fin + 1, :, ds(b2 * NPC, NPC)],
                        agg_nodes[r : r + T, :],
                    )

            # ---------------- Stage 2: packed GRU, 2 column streams ----------------
            with (
                tc.tile_pool(name="work", bufs=2) as wp,
                tc.tile_pool(name="scratch", bufs=1) as sp,
                tc.tile_pool(name="pzrz", bufs=2, space="PSUM") as pzrz_pool,
                tc.tile_pool(name="pht", bufs=2, space="PSUM") as pht_pool,
                tc.tile_pool(name="plin", bufs=1, space="PSUM") as plin_pool,
            ):
                tiles = {}

                # 512-col chunks across the packed width; grouped in pairs so
                # consecutive matmuls share lhsT (weight switches cost ~400cyc
                # of PE fill/drain; same-weight matmuls pipeline at ~N cycles)
                ZCH = [(0, 512), (512, 512), (1024, 512), (1536, 512), (2048, 452)]
                ZPAIRS = [[0, 1], [2, 3], [4]]

                def emit_zr_sig_rh(t):
                    h_prev = h_bufs[t % 2]
                    rz = wp.tile([P, 2, W], BF16, tag="rz", name="rz")
                    rh = sp.tile([P, W], BF16, tag="rh", name="rh")
                    tiles[("rz", t)], tiles[("rh", t)] = rz, rh
                    for pair in ZPAIRS:
                        pzs = []
                        for ci in pair:
                            pzs.append(pzrz_pool.tile([P, 2, 512], F32, tag="pz", name="pz"))
                        for gate, wg in ((0, wR), (1, wZ)):
                            for pz, ci in zip(pzs, pair):
                                c0, cw = ZCH[ci]
                                nc.tensor.matmul(pz[:, gate, :cw], lhsT=wg[:],
                                                 rhs=h_prev[:, ds(c0, cw)],
                                                 start=True, stop=False)
                        for gate, ag in ((0, aR), (1, aZ)):
                            for pz, ci in zip(pzs, pair):
                                c0, cw = ZCH[ci]
                                nc.tensor.matmul(pz[:, gate, :cw], lhsT=ag[:],
                                                 rhs=aggp[:, t, ds(c0, cw)],
                                                 start=False, stop=True)
                        for pz, ci in zip(pzs, pair):
                            c0, cw = ZCH[ci]
                            nc.scalar.activation(rz[:, :, ds(c0, cw)], pz[:, :, :cw],
                                                 AF.Sigmoid)
                    for s in range(2):
                        sc = ds(s * SW, SW)
                        nc.vector.tensor_tensor(rh[:, sc], in0=rz[:, 0, sc],
                                                in1=h_prev[:, sc], op=ALU.mult)

                def emit_h(t):
                    rh = tiles[("rh", t)]
                    ht = sp.tile([P, W], BF16, tag="ht", name="ht")
                    tiles[("ht", t)] = ht
                    for pair in ZPAIRS:
                        phs = [pht_pool.tile([P, 512], F32, tag="ph", name="ph")
                               for _ in pair]
                        for ph, ci in zip(phs, pair):
                            c0, cw = ZCH[ci]
                            nc.tensor.matmul(ph[:, :cw], lhsT=wH[:],
                                             rhs=rh[:, ds(c0, cw)],
                                             start=True, stop=False)
                        for ph, ci in zip(phs, pair):
                            c0, cw = ZCH[ci]
                            nc.tensor.matmul(ph[:, :cw], lhsT=aH[:],
                                             rhs=aggp[:, t, ds(c0, cw)],
                                             start=False, stop=True)
                        for ph, ci in zip(phs, pair):
                            c0, cw = ZCH[ci]
                            nc.scalar.activation(ht[:, ds(c0, cw)], ph[:, :cw], AF.Tanh)

                def emit_hn(t):
                    h_prev, h_next = h_bufs[t % 2], h_bufs[(t + 1) % 2]
                    rz, ht = tiles.pop(("rz", t)), tiles.pop(("ht", t))
                    rl = sp.tile([P, W], BF16, tag="rl", name="rl")
                    tiles[("rl", t)] = rl
                    dd = sp.tile([P, W], BF16, tag="dd", name="dd")
                    zd = sp.tile([P, W], BF16, tag="zd", name="zd")
                    for s in range(2):
                        sc = ds(s * SW, SW)
                        nc.vector.tensor_tensor(dd[:, sc], in0=ht[:, sc],
                                                in1=h_prev[:, sc], op=ALU.subtract)
                        nc.vector.tensor_tensor(zd[:, sc], in0=rz[:, 1, sc],
                                                in1=dd[:, sc], op=ALU.mult)
                        nc.vector.tensor_tensor(h_next[:, sc], in0=ht[:, sc],
                                                in1=zd[:, sc], op=ALU.subtract)
                        nc.vector.tensor_scalar_max(rl[:, sc], h_next[:, sc], 0.0)
                    tiles.pop(("rh", t))

                def emit_lin(t):
                    rl = tiles.pop(("rl", t))
                    lo = sp.tile([P, LG], F32, tag="lo", name="lo")
                    pl = plin_pool.tile([P, LG], F32, tag="pl", name="pl")
                    for g in range(4):
                        for so, sw in [(0, 512), (512, LG - 512)]:
                            nc.tensor.matmul(pl[32 * g : 32 * g + 32, ds(so, sw)],
                                             lhsT=wL[:],
                                             rhs=rl[:, ds(g * LG + so, sw)],
                                             start=True, stop=True,
                                             tile_position=(0, 32 * g))
                    nc.vector.tensor_copy(lo[:], pl[:])
                    nc.sync.dma_start(out_d[t], lo[:])

                for t in range(T):
                    emit_zr_sig_rh(t)
                    if t > 0:
                        emit_lin(t - 1)       # fills PE while ACT runs sigmoid(t)
                    emit_h(t)
                    emit_hn(t)
                emit_lin(T - 1)

    nc.compile()
    return nc


def _prep_host(x, edge_index, edge_weight, Wz, bz, Wr, br, Wh, bh,
               Lz_w, Lz_b, Lr_w, Lr_b, Lh_w, Lh_b, lin_w, lin_b):
    """Host-side preprocessing: norm, dense fp8 A, packed/folded weights."""
    bf16 = ml_dtypes.bfloat16
    fp8 = ml_dtypes.float8_e3m4
    f32 = np.float32

    src = np.asarray(edge_index[0], dtype=np.int64)
    dst = np.asarray(edge_index[1], dtype=np.int64)
    w = np.asarray(edge_weight, dtype=f32)
    loop = np.arange(N_NODES, dtype=np.int64)
    src_a = np.concatenate([src, loop])
    dst_a = np.concatenate([dst, loop])
    w_a = np.concatenate([w, np.ones(N_NODES, f32)])
    deg = np.zeros(N_NODES, f32)
    np.add.at(deg, dst_a, w_a)
    dinv = np.where(deg > 0, 1.0 / np.sqrt(deg), 0.0).astype(f32)
    norm = dinv[src_a] * w_a * dinv[dst_a]

    A = np.zeros((N_SRC_PAD, N_NODES), f32)   # A[src, dst]
    np.add.at(A, (src_a, dst_a), norm)
    A_f8 = (A * A_SCALE).astype(fp8)

    # X rows: [n_src_pad, 96], f = b*24 + fin*12 + t; scaled by 1/A_SCALE
    X = np.zeros((N_SRC_PAD, NFEAT), f32)
    X[:N_NODES] = np.transpose(np.asarray(x, f32), (1, 0, 2, 3)).reshape(N_NODES, -1)
    X *= 1.0 / A_SCALE
    x_feat = np.ascontiguousarray(
        X.astype(bf16).reshape(KCH, P, NFEAT).transpose(1, 0, 2))

    Wz, Wr, Wh = np.asarray(Wz, f32), np.asarray(Wr, f32), np.asarray(Wh, f32)
    Lz_w, Lr_w, Lh_w = np.asarray(Lz_w, f32), np.asarray(Lr_w, f32), np.asarray(Lh_w, f32)
    Az, Ar, Ah = Wz @ Lz_w[:HID], Wr @ Lr_w[:HID], Wh @ Lh_w[:HID]   # [2, 64]
    Lz2, Lr2, Lh2 = Lz_w[HID:], Lr_w[HID:], Lh_w[HID:]               # [64, 64]
    bz_f = np.asarray(bz, f32) @ Lz_w[:HID] + np.asarray(Lz_b, f32)
    br_f = np.asarray(br, f32) @ Lr_w[:HID] + np.asarray(Lr_b, f32)
    bh_f = np.asarray(bh, f32) @ Lh_w[:HID] + np.asarray(Lh_b, f32)

    def blockdiag(M):
        out = np.zeros((P, P), f32)
        out[:HID, :HID] = M
        out[HID:, HID:] = M
        return out

    def aggw(Am, bias):
        out = np.zeros((5, P), f32)
        for h in range(2):
            out[2 * h : 2 * h + 2, HID * h : HID * h + HID] = Am
        out[4, :HID] = bias
        out[4, HID:] = bias
        return out

    wlin = np.zeros((P, 32), f32)
    wlin[:HID, :OUT] = np.asarray(lin_w, f32)
    wlin[HID:, OUT:] = np.asarray(lin_w, f32)

    common = {
        "x_feat": x_feat,
        "lr2bd": blockdiag(Lr2).astype(bf16),
        "lz2bd": blockdiag(Lz2).astype(bf16),
        "lh2bd": blockdiag(Lh2).astype(bf16),
        "agr": aggw(Ar, br_f).astype(bf16),
        "agz": aggw(Az, bz_f).astype(bf16),
        "agh": aggw(Ah, bh_f).astype(bf16),
        "wlin": wlin.astype(bf16),
        "ones_d": np.ones((1, T * W), f32).astype(bf16),
    }
    in_maps = []
    for c in range(N_CORES):
        a_core = np.ascontiguousarray(
            A_f8[:, c * NPC : (c + 1) * NPC].reshape(KCH, P, NPC).transpose(1, 0, 2))
        in_maps.append(dict(common, a_t=a_core))
    return in_maps


_CACHED_NC = None


def kernel(**inputs) -> np.ndarray:
    global _CACHED_NC
    from concourse.bass_utils import run_bass_kernel_spmd

    in_maps = _prep_host(**inputs)
    if _CACHED_NC is None:
        _CACHED_NC = build_program()
    res = run_bass_kernel_spmd(_CACHED_NC, in_maps, core_ids=list(range(N_CORES)))

    lin_b = np.asarray(inputs["lin_b"], np.float32)
    full = np.empty((B, T, N_NODES, OUT), np.float32)
    for c, r in enumerate(res.results):
        o = r["out"]                                  # [T, 128, 625]
        # row 32g+16h+o_, col cc -> packed col j=625g+cc; b2=j//1250; n=j%1250
        o = o.reshape(T, 4, 2, OUT, LG)               # [t, g, h, o, cc]
        for g in range(4):
            b2, n0 = g // 2, (g % 2) * LG
            for h in range(2):
                b = 2 * h + b2
                full[b, :, c * NPC + n0 : c * NPC + n0 + LG, :] = \
                    o[:, g, h, :, :].transpose(0, 2, 1)
    return full + lin_b
